# revision 3
# baseline (speedup 1.0000x reference)
"""PraxisSMEAR (soft-merged MoE GLU MLP) on 8 TRN2 NeuronCores.

Strategy: data-parallel over tokens (1024 per core).  The expert weight
merge (weighted sum over E=8 experts) is sharded over the contraction
dims (d for W_up, f for W_dn), computed on the TensorEngine via a
stacked-scaled-identity matmul, and the merged bf16 weights are
AllGathered.  Router runs on-device with a tiny [4,8] AllReduce of
partial logits.  Main GEMMs run in bf16 with f32 PSUM accumulation.

Host-side work is limited to sharding/relayout: transposes, an
o-permutation of W_up rows (so linear/gated GLU tiles are adjacent),
f32->bf16 casts, and the final transpose-concat of per-core outputs.
"""

import numpy as np
import ml_dtypes

from concourse import bacc, tile
import concourse.bass as bass
import concourse.mybir as mybir
from concourse.bass import ds, ts
from concourse.bass_utils import run_bass_kernel_spmd
from concourse.masks import make_identity

NCORES = 8
P = 128
B, S, D = 4, 2048, 1024
T = B * S               # 8192 tokens total
TPC = T // NCORES       # 1024 tokens per core
E = 8
DFF = 8 * D             # 8192
F = 4 * D               # 4096 (GLU half)
KS = D // P             # 8 k-subtiles of 128 over d
FKS = F // P            # 32 k-subtiles of 128 over f
RWB_ROWS = D + P        # router_w.T rows padded: row D = router_b

F32 = mybir.dt.float32
BF16 = mybir.dt.bfloat16

_CACHED = {}


def _build():
    nc = bacc.Bacc("TRN2", target_bir_lowering=False, debug=False,
                   num_devices=NCORES)

    xt = nc.dram_tensor("xt", [D, TPC], F32, kind="ExternalInput")
    rwb = nc.dram_tensor("rwb", [RWB_ROWS, E], F32, kind="ExternalInput")
    upw = nc.dram_tensor("upw", [E, P, DFF], BF16, kind="ExternalInput")
    dnw = nc.dram_tensor("dnw", [E, 4, P, D], BF16, kind="ExternalInput")
    upb = nc.dram_tensor("upb", [E, DFF], F32, kind="ExternalInput")
    dnb = nc.dram_tensor("dnb", [E, D], F32, kind="ExternalInput")
    bsel = nc.dram_tensor("bsel", [P, B], F32, kind="ExternalInput")
    out = nc.dram_tensor("out", [D, TPC], F32, kind="ExternalOutput")

    rg = [list(range(NCORES))]

    with tile.TileContext(nc) as tc:
        with (
            tc.tile_pool(name="const", bufs=1) as const,
            tc.tile_pool(name="dram", bufs=1, space="DRAM") as dram,
        ):
            # ---- internal DRAM (collective bounce buffers) ----
            ar_in = dram.tile([B, E], F32)
            ar_out = dram.tile([B, E], F32, addr_space="Shared")
            agu_in = dram.tile([P, DFF], BF16)
            agu_out = dram.tile([NCORES * P, DFF], BF16, addr_space="Shared")
            agd_in = dram.tile([F // NCORES, D], BF16)
            agd_out = dram.tile([F, D], BF16, addr_space="Shared")
            bup_dram = dram.tile([DFF], F32)
            bdn_dram = dram.tile([D], F32)

            # ---- persistent SBUF ----
            xt_bf = const.tile([P, KS, TPC], BF16)
            st_all = const.tile([P, FKS, TPC], BF16)
            id_bf = const.tile([P, P], BF16)
            make_identity(nc, id_bf)
            id_f32 = const.tile([P, P], F32)
            make_identity(nc, id_f32)
            id_scaled = const.tile([P, E, P], BF16)
            probs_bcast = const.tile([P, E], F32)
            probs_col = const.tile([P, 1], F32)
            bup_sb = const.tile([P, DFF // P], F32)
            bdn_sb = const.tile([P, D // P], F32)
            bsel_sb = const.tile([P, B], F32)
            e0_ones = const.tile([P, P], F32)
            nc.vector.memset(e0_ones[:], 0.0)
            nc.vector.memset(e0_ones[0:1, :], 1.0)
            quarter = const.tile([P, 1], F32)
            nc.vector.memset(quarter[:], 0.0)
            nc.vector.memset(quarter[0:B, :], 1.0 / B)

            nc.sync.dma_start(bsel_sb[:], bsel[:])

            # ================= router =================
            with (
                tc.tile_pool(name="rstage", bufs=3) as rstage,
                tc.tile_pool(name="rmisc", bufs=1) as rmisc,
                tc.tile_pool(name="rpsum", bufs=1, space="PSUM") as rpsum,
            ):
                red4 = rmisc.tile([P, KS, 4], F32)
                NCH = TPC // 4
                for j in range(4):
                    xs = rstage.tile([P, KS, NCH], F32)
                    nc.sync.dma_start(
                        xs[:],
                        xt.rearrange("(ks p) t -> p ks t", p=P)[:, :, ds(j * NCH, NCH)],
                    )
                    nc.vector.tensor_copy(xt_bf[:, :, ds(j * NCH, NCH)], xs[:])
                    nc.vector.reduce_sum(red4[:, :, j:j + 1], xs[:],
                                         axis=mybir.AxisListType.X)

                red = rmisc.tile([P, KS + 1, 1], F32)
                nc.vector.reduce_sum(red[:, :KS, :], red4[:],
                                     axis=mybir.AxisListType.X)
                nc.vector.tensor_scalar_mul(red[:, :KS, :], red[:, :KS, :], 1.0 / S)
                # bias row: after the 8-core AllReduce each batch gets 2
                # contributions, so 0.5 * router_b * 2 = router_b
                nc.vector.memset(red[:, KS, :], 0.0)
                nc.vector.memset(red[0:1, KS, :], 0.5)

                sel = rmisc.tile([P, KS + 1, B], F32)
                for kt in range(KS + 1):
                    nc.vector.tensor_tensor(
                        sel[:, kt, :],
                        red[:, kt, :].to_broadcast([P, B]),
                        bsel_sb[:],
                        mybir.AluOpType.mult,
                    )

                rwb_sb = rmisc.tile([P, KS + 1, E], F32)
                nc.sync.dma_start(rwb_sb[:], rwb.rearrange("(ks p) e -> p ks e", p=P))

                ps_l = rpsum.tile([B, E], F32)
                for kt in range(KS + 1):
                    nc.tensor.matmul(ps_l[:], sel[:, kt, :], rwb_sb[:, kt, :],
                                     start=(kt == 0), stop=(kt == KS))
                logits_part = rmisc.tile([B, E], F32)
                nc.vector.tensor_copy(logits_part[:], ps_l[:])
                nc.sync.dma_start(ar_in[:], logits_part[:])
                nc.gpsimd.collective_compute(
                    "AllReduce", mybir.AluOpType.add, replica_groups=rg,
                    ins=[ar_in[:]], outs=[ar_out[:]],
                )
                logits = rmisc.tile([B, E], F32)
                nc.sync.dma_start(logits[:], ar_out[:])

                # softmax over E per batch, then mean over batches
                mx = rmisc.tile([B, 1], F32)
                nc.vector.reduce_max(mx[:], logits[:], axis=mybir.AxisListType.X)
                negmx = rmisc.tile([B, 1], F32)
                nc.vector.tensor_scalar_mul(negmx[:], mx[:], -1.0)
                probs_pad = rmisc.tile([P, E], F32)
                nc.vector.memset(probs_pad[:], 0.0)
                nc.scalar.activation(probs_pad[0:B, :], logits[:],
                                     mybir.ActivationFunctionType.Exp,
                                     bias=negmx[:], scale=1.0)
                sm = rmisc.tile([B, 1], F32)
                nc.vector.reduce_sum(sm[:], probs_pad[0:B, :],
                                     axis=mybir.AxisListType.X)
                rc = rmisc.tile([B, 1], F32)
                nc.vector.reciprocal(rc[:], sm[:])
                nc.vector.tensor_scalar_mul(probs_pad[0:B, :], probs_pad[0:B, :],
                                            rc[:])

                ps_pm = rpsum.tile([1, E], F32)
                nc.tensor.matmul(ps_pm[:], quarter[:], probs_pad[:],
                                 start=True, stop=True)
                prow_pad = rmisc.tile([P, E], F32)
                nc.vector.memset(prow_pad[:], 0.0)
                nc.vector.tensor_copy(prow_pad[0:1, :], ps_pm[:])
                ps_pb = rpsum.tile([P, E], F32)
                nc.tensor.matmul(ps_pb[:], e0_ones[:], prow_pad[:],
                                 start=True, stop=True)
                nc.vector.tensor_copy(probs_bcast[:], ps_pb[:])

                # scaled identities for the merge matmuls
                for e in range(E):
                    nc.vector.tensor_scalar_mul(id_scaled[:, e, :], id_bf[:],
                                                probs_bcast[:, e:e + 1])
                # probs as a column vector (rows 0..7), for bias merges
                nc.vector.memset(probs_col[:], 0.0)
                tmp8 = rmisc.tile([E, E], F32)
                nc.vector.tensor_tensor(tmp8[:], probs_bcast[0:E, :],
                                        id_f32[0:E, 0:E], mybir.AluOpType.mult)
                nc.vector.reduce_sum(probs_col[0:E, :], tmp8[:],
                                     axis=mybir.AxisListType.X)

            # ================= weight merge (PE) + AllGather =================
            with (
                tc.tile_pool(name="merge", bufs=3) as merge,
                tc.tile_pool(name="mev", bufs=3) as mev,
                tc.tile_pool(name="mpsum", bufs=2, space="PSUM") as mpsum,
            ):
                CH = 512
                for j in range(DFF // CH):
                    win = merge.tile([P, E, CH], BF16)
                    nc.sync.dma_start(
                        win[:],
                        upw[:, :, ds(j * CH, CH)].rearrange("e p o -> p e o"),
                    )
                    ps = mpsum.tile([P, CH], F32)
                    for e in range(E):
                        nc.tensor.matmul(ps[:], id_scaled[:, e, :], win[:, e, :],
                                         start=(e == 0), stop=(e == E - 1))
                    ev = mev.tile([P, CH], BF16)
                    nc.vector.tensor_copy(ev[:], ps[:])
                    nc.sync.dma_start(agu_in[:, ds(j * CH, CH)], ev[:])

                nc.gpsimd.collective_compute(
                    "AllGather", mybir.AluOpType.bypass, replica_groups=rg,
                    ins=[agu_in[:]], outs=[agu_out[:]],
                )

                for fq in range(4):
                    for h in range(D // CH):
                        win = merge.tile([P, E, CH], BF16)
                        nc.sync.dma_start(
                            win[:],
                            dnw[:, fq, :, ds(h * CH, CH)].rearrange("e p n -> p e n"),
                        )
                        ps = mpsum.tile([P, CH], F32)
                        for e in range(E):
                            nc.tensor.matmul(ps[:], id_scaled[:, e, :],
                                             win[:, e, :],
                                             start=(e == 0), stop=(e == E - 1))
                        ev = mev.tile([P, CH], BF16)
                        nc.vector.tensor_copy(ev[:], ps[:])
                        nc.sync.dma_start(
                            agd_in[ds(fq * P, P), ds(h * CH, CH)], ev[:])

                nc.gpsimd.collective_compute(
                    "AllGather", mybir.AluOpType.bypass, replica_groups=rg,
                    ins=[agd_in[:]], outs=[agd_out[:]],
                )

            # ================= bias merges =================
            with (
                tc.tile_pool(name="bias", bufs=2) as bias,
                tc.tile_pool(name="bpsum", bufs=2, space="PSUM") as bpsum,
            ):
                CH = 512
                for j in range(DFF // CH):
                    ub = bias.tile([P, CH], F32)
                    nc.vector.memset(ub[:], 0.0)
                    nc.sync.dma_start(ub[0:E, :], upb[:, ds(j * CH, CH)])
                    pb = bpsum.tile([1, CH], F32)
                    nc.tensor.matmul(pb[:], probs_col[:], ub[:],
                                     start=True, stop=True)
                    bev = bias.tile([1, CH], F32)
                    nc.vector.tensor_copy(bev[:], pb[:])
                    nc.sync.dma_start(bup_dram[None, ds(j * CH, CH)], bev[:])
                for j in range(D // CH):
                    db = bias.tile([P, CH], F32)
                    nc.vector.memset(db[:], 0.0)
                    nc.sync.dma_start(db[0:E, :], dnb[:, ds(j * CH, CH)])
                    pb = bpsum.tile([1, CH], F32)
                    nc.tensor.matmul(pb[:], probs_col[:], db[:],
                                     start=True, stop=True)
                    bev = bias.tile([1, CH], F32)
                    nc.vector.tensor_copy(bev[:], pb[:])
                    nc.sync.dma_start(bdn_dram[None, ds(j * CH, CH)], bev[:])

                nc.sync.dma_start(bup_sb[:],
                                  bup_dram.rearrange("(ot p) -> p ot", p=P))
                nc.sync.dma_start(bdn_sb[:],
                                  bdn_dram.rearrange("(dt p) -> p dt", p=P))

            # ================= up GEMM + GLU -> st_all =================
            # o' layout (host-permuted): o'-tile 2t = linear f-tile t,
            # o'-tile 2t+1 = gated f-tile t.
            agu_view = agu_out.rearrange("(kt p) o -> p kt o", p=P)
            TH = TPC // 512
            with (
                tc.tile_pool(name="wstrip", bufs=3) as wstrip,
                tc.tile_pool(name="glu", bufs=3) as glu,
                tc.tile_pool(name="gpsum", bufs=2, space="PSUM") as gpsum,
            ):
                for oc in range(DFF // 512):
                    wsu = wstrip.tile([P, KS, 512], BF16)
                    nc.sync.dma_start(wsu[:], agu_view[:, :, ds(oc * 512, 512)])
                    for j2 in range(2):
                        ft = 2 * oc + j2
                        lhs_l = wsu[:, :, ds(j2 * 256, P)]
                        lhs_g = wsu[:, :, ds(j2 * 256 + P, P)]
                        ps_l0 = gpsum.tile([P, 512], F32, tag="ps_l0")
                        ps_l1 = gpsum.tile([P, 512], F32, tag="ps_l1")
                        ps_g0 = gpsum.tile([P, 512], F32, tag="ps_g0")
                        ps_g1 = gpsum.tile([P, 512], F32, tag="ps_g1")
                        for kt in range(KS):
                            nc.tensor.matmul(ps_l0[:], lhs_l[:, kt, :],
                                             xt_bf[:, kt, 0:512],
                                             start=(kt == 0), stop=(kt == KS - 1))
                            nc.tensor.matmul(ps_l1[:], lhs_l[:, kt, :],
                                             xt_bf[:, kt, 512:1024],
                                             start=(kt == 0), stop=(kt == KS - 1))
                        for kt in range(KS):
                            nc.tensor.matmul(ps_g0[:], lhs_g[:, kt, :],
                                             xt_bf[:, kt, 0:512],
                                             start=(kt == 0), stop=(kt == KS - 1))
                            nc.tensor.matmul(ps_g1[:], lhs_g[:, kt, :],
                                             xt_bf[:, kt, 512:1024],
                                             start=(kt == 0), stop=(kt == KS - 1))
                        for th, ps_lx, ps_gx in ((0, ps_l0, ps_g0),
                                                 (1, ps_l1, ps_g1)):
                            lin = glu.tile([P, 512], F32, tag="lin")
                            nc.vector.tensor_scalar_add(
                                lin[:], ps_lx[:], bup_sb[:, 2 * ft:2 * ft + 1])
                            sil = glu.tile([P, 512], F32, tag="sil")
                            nc.scalar.activation(
                                sil[:], ps_gx[:],
                                mybir.ActivationFunctionType.Silu,
                                bias=bup_sb[:, 2 * ft + 1:2 * ft + 2], scale=1.0)
                            nc.vector.tensor_tensor(
                                st_all[:, ft, ds(th * 512, 512)], lin[:], sil[:],
                                mybir.AluOpType.mult)

            # ================= down GEMM =================
            agd_view = agd_out.rearrange("(kt p) n -> p kt n", p=P)
            with (
                tc.tile_pool(name="dwstrip", bufs=2) as dwstrip,
                tc.tile_pool(name="opool", bufs=3) as opool,
                tc.tile_pool(name="dpsum", bufs=2, space="PSUM") as dpsum,
            ):
                for dp in range(4):
                    wd = dwstrip.tile([P, FKS, 256], BF16)
                    nc.sync.dma_start(wd[:], agd_view[:, :, ds(dp * 256, 256)])
                    for dtl in range(2):
                        dt = 2 * dp + dtl
                        ps_o0 = dpsum.tile([P, 512], F32, tag="ps_o0")
                        ps_o1 = dpsum.tile([P, 512], F32, tag="ps_o1")
                        for kt in range(FKS):
                            nc.tensor.matmul(ps_o0[:], wd[:, kt, ds(dtl * P, P)],
                                             st_all[:, kt, 0:512],
                                             start=(kt == 0), stop=(kt == FKS - 1))
                            nc.tensor.matmul(ps_o1[:], wd[:, kt, ds(dtl * P, P)],
                                             st_all[:, kt, 512:1024],
                                             start=(kt == 0), stop=(kt == FKS - 1))
                        for th, ps_ox in ((0, ps_o0), (1, ps_o1)):
                            ot = opool.tile([P, 512], F32, tag="ot")
                            nc.vector.tensor_scalar_add(
                                ot[:], ps_ox[:], bdn_sb[:, dt:dt + 1])
                            nc.sync.dma_start(
                                out[ds(dt * P, P), ds(th * 512, 512)], ot[:])

    nc.compile()
    return nc


def _get_nc():
    if "nc" not in _CACHED:
        _CACHED["nc"] = _build()
    return _CACHED["nc"]


def _prep(x, router_w, router_b, up_w, up_b, down_w, down_b):
    x = np.asarray(x, dtype=np.float32)
    router_w = np.asarray(router_w, dtype=np.float32)
    router_b = np.asarray(router_b, dtype=np.float32)
    up_w = np.asarray(up_w, dtype=np.float32)
    up_b = np.asarray(up_b, dtype=np.float32)
    down_w = np.asarray(down_w, dtype=np.float32)
    down_b = np.asarray(down_b, dtype=np.float32)

    bf = ml_dtypes.bfloat16

    # o-permutation: [L0, G0, L1, G1, ...] blocks of 128 rows
    perm = np.empty(DFF, dtype=np.int64)
    for t in range(F // P):
        perm[2 * t * P:(2 * t + 1) * P] = np.arange(t * P, (t + 1) * P)
        perm[(2 * t + 1) * P:(2 * t + 2) * P] = np.arange(F + t * P, F + (t + 1) * P)

    xf = x.reshape(T, D)
    up_bf = up_w.astype(bf)[:, perm, :]          # [E, DFF(perm), D]
    dn_bf = down_w.astype(bf)                    # [E, D, F]

    rwb = np.zeros((RWB_ROWS, E), dtype=np.float32)
    rwb[:D] = router_w.T
    rwb[D] = router_b
    upb_p = np.ascontiguousarray(up_b[:, perm])
    dnb_c = np.ascontiguousarray(down_b)

    in_maps = []
    for c in range(NCORES):
        xt_c = np.ascontiguousarray(xf[c * TPC:(c + 1) * TPC].T)
        upw_c = np.ascontiguousarray(
            up_bf[:, :, c * P:(c + 1) * P].transpose(0, 2, 1))   # [E, 128, DFF]
        dnw_c = np.ascontiguousarray(
            dn_bf[:, :, c * (F // NCORES):(c + 1) * (F // NCORES)]
            .transpose(0, 2, 1)).reshape(E, 4, P, D)             # [E, 4, 128, D]
        bsel_c = np.zeros((P, B), dtype=np.float32)
        bsel_c[:, c // 2] = 1.0
        in_maps.append({
            "xt": xt_c, "rwb": rwb, "upw": upw_c, "dnw": dnw_c,
            "upb": upb_p, "dnb": dnb_c, "bsel": bsel_c,
        })
    return in_maps


def kernel(x, router_w, router_b, up_w, up_b, down_w, down_b):
    in_maps = _prep(x, router_w, router_b, up_w, up_b, down_w, down_b)
    nc = _get_nc()
    res = run_bass_kernel_spmd(nc, in_maps, core_ids=list(range(NCORES)))

    outp = np.empty((T, D), dtype=np.float32)
    for c in range(NCORES):
        outp[c * TPC:(c + 1) * TPC] = res.results[c]["out"].T
    return outp.reshape(B, S, D)


# revision 5
# speedup vs baseline: 1.1093x; 1.1093x over previous
"""PraxisSMEAR (soft-merged MoE GLU MLP) on 8 TRN2 NeuronCores.

Strategy: data-parallel over tokens (1024 per core).  The expert weight
merge (weighted sum over E=8 experts) is sharded over the contraction
dims (d for W_up, f for W_dn), computed on the TensorEngine via a
stacked-scaled-identity matmul, and the merged bf16 weights are
AllGathered in 8 o-chunks so the up GEMM starts as soon as the first
chunk lands.  Router runs on-device with a tiny [4,8] AllReduce of
partial logits.  Main GEMMs run in bf16 with f32 PSUM accumulation.

Host-side work is limited to sharding/relayout: transposes, an
o-permutation of W_up rows (so linear/gated GLU tiles are adjacent),
f32->bf16 casts, and the final transpose-concat of per-core outputs.
"""

import numpy as np
import ml_dtypes

from concourse import bacc, tile
import concourse.bass as bass
import concourse.mybir as mybir
from concourse.bass import ds, ts
from concourse.bass_utils import run_bass_kernel_spmd
from concourse.masks import make_identity

NCORES = 8
P = 128
B, S, D = 4, 2048, 1024
T = B * S               # 8192 tokens total
TPC = T // NCORES       # 1024 tokens per core
E = 8
DFF = 8 * D             # 8192
F = 4 * D               # 4096 (GLU half)
KS = D // P             # 8 k-subtiles of 128 over d
FKS = F // P            # 32 k-subtiles of 128 over f
RWB_ROWS = D + P        # router_w.T rows padded: row D = router_b
NAG = 8                 # AllGather chunks over the o' axis (1024 cols each)

F32 = mybir.dt.float32
BF16 = mybir.dt.bfloat16

_CACHED = {}


def _build():
    nc = bacc.Bacc("TRN2", target_bir_lowering=False, debug=False,
                   num_devices=NCORES)

    xt = nc.dram_tensor("xt", [D, TPC], F32, kind="ExternalInput")
    rwb = nc.dram_tensor("rwb", [RWB_ROWS, E], F32, kind="ExternalInput")
    upw = nc.dram_tensor("upw", [DFF // 512, P, E, 512], BF16,
                         kind="ExternalInput")
    dnw = nc.dram_tensor("dnw", [4, 2, P, E, 512], BF16, kind="ExternalInput")
    upb = nc.dram_tensor("upb", [E, DFF], F32, kind="ExternalInput")
    dnb = nc.dram_tensor("dnb", [E, D], F32, kind="ExternalInput")
    bsel = nc.dram_tensor("bsel", [P, B], F32, kind="ExternalInput")
    out = nc.dram_tensor("out", [D, TPC], F32, kind="ExternalOutput")

    rg = [list(range(NCORES))]

    with tile.TileContext(nc) as tc:
        with (
            tc.tile_pool(name="const", bufs=1) as const,
            tc.tile_pool(name="dram", bufs=1, space="DRAM") as dram,
        ):
            # ---- internal DRAM (collective bounce buffers) ----
            ar_in = dram.tile([B, E], F32)
            ar_out = dram.tile([B, E], F32, addr_space="Shared")
            agu_in = [dram.tile([P, 1024], BF16, name=f"agu_in{i}")
                      for i in range(NAG)]
            agu_out = [dram.tile([NCORES * P, 1024], BF16, addr_space="Shared",
                                 name=f"agu_out{i}")
                       for i in range(NAG)]
            agd_in = dram.tile([F // NCORES, D], BF16)
            agd_out = dram.tile([F, D], BF16, addr_space="Shared")
            bup_dram = dram.tile([DFF], F32)
            bdn_dram = dram.tile([D], F32)

            # ---- persistent SBUF ----
            xt_bf = const.tile([P, KS, TPC], BF16)
            id_bf = const.tile([P, P], BF16)
            make_identity(nc, id_bf)
            id_f32 = const.tile([P, P], F32)
            make_identity(nc, id_f32)
            id_scaled = const.tile([P, E, P], BF16)
            probs_bcast = const.tile([P, E], F32)
            probs_col = const.tile([P, 1], F32)
            bup_sb = const.tile([P, DFF // P], F32)
            bdn_sb = const.tile([P, D // P], F32)
            bsel_sb = const.tile([P, B], F32)
            e0_ones = const.tile([P, P], F32)
            nc.vector.memset(e0_ones[:], 0.0)
            nc.vector.memset(e0_ones[0:1, :], 1.0)
            quarter = const.tile([P, 1], F32)
            nc.vector.memset(quarter[:], 0.0)
            nc.vector.memset(quarter[0:B, :], 1.0 / B)

            nc.sync.dma_start(bsel_sb[:], bsel[:])

            # ================= router =================
            with (
                tc.tile_pool(name="rstage", bufs=3) as rstage,
                tc.tile_pool(name="rmisc", bufs=1) as rmisc,
                tc.tile_pool(name="rpsum", bufs=1, space="PSUM") as rpsum,
            ):
                red4 = rmisc.tile([P, KS, 4], F32)
                NCH = TPC // 4
                for j in range(4):
                    xs = rstage.tile([P, KS, NCH], F32)
                    nc.sync.dma_start(
                        xs[:],
                        xt.rearrange("(ks p) t -> p ks t", p=P)[:, :, ds(j * NCH, NCH)],
                    )
                    nc.vector.tensor_copy(xt_bf[:, :, ds(j * NCH, NCH)], xs[:])
                    nc.vector.reduce_sum(red4[:, :, j:j + 1], xs[:],
                                         axis=mybir.AxisListType.X)

                red = rmisc.tile([P, KS + 1, 1], F32)
                nc.vector.reduce_sum(red[:, :KS, :], red4[:],
                                     axis=mybir.AxisListType.X)
                nc.vector.tensor_scalar_mul(red[:, :KS, :], red[:, :KS, :], 1.0 / S)
                # bias row: after the 8-core AllReduce each batch gets 2
                # contributions, so 0.5 * router_b * 2 = router_b
                nc.vector.memset(red[:, KS, :], 0.0)
                nc.vector.memset(red[0:1, KS, :], 0.5)

                sel = rmisc.tile([P, KS + 1, B], F32)
                for kt in range(KS + 1):
                    nc.vector.tensor_tensor(
                        sel[:, kt, :],
                        red[:, kt, :].to_broadcast([P, B]),
                        bsel_sb[:],
                        mybir.AluOpType.mult,
                    )

                rwb_sb = rmisc.tile([P, KS + 1, E], F32)
                nc.sync.dma_start(rwb_sb[:], rwb.rearrange("(ks p) e -> p ks e", p=P))

                ps_l = rpsum.tile([B, E], F32)
                for kt in range(KS + 1):
                    nc.tensor.matmul(ps_l[:], sel[:, kt, :], rwb_sb[:, kt, :],
                                     start=(kt == 0), stop=(kt == KS))
                logits_part = rmisc.tile([B, E], F32)
                nc.vector.tensor_copy(logits_part[:], ps_l[:])
                nc.sync.dma_start(ar_in[:], logits_part[:])
                nc.gpsimd.collective_compute(
                    "AllReduce", mybir.AluOpType.add, replica_groups=rg,
                    ins=[ar_in[:]], outs=[ar_out[:]],
                )
                logits = rmisc.tile([B, E], F32)
                nc.sync.dma_start(logits[:], ar_out[:])

                # softmax over E per batch, then mean over batches
                mx = rmisc.tile([B, 1], F32)
                nc.vector.reduce_max(mx[:], logits[:], axis=mybir.AxisListType.X)
                negmx = rmisc.tile([B, 1], F32)
                nc.vector.tensor_scalar_mul(negmx[:], mx[:], -1.0)
                probs_pad = rmisc.tile([P, E], F32)
                nc.vector.memset(probs_pad[:], 0.0)
                nc.scalar.activation(probs_pad[0:B, :], logits[:],
                                     mybir.ActivationFunctionType.Exp,
                                     bias=negmx[:], scale=1.0)
                sm = rmisc.tile([B, 1], F32)
                nc.vector.reduce_sum(sm[:], probs_pad[0:B, :],
                                     axis=mybir.AxisListType.X)
                rc = rmisc.tile([B, 1], F32)
                nc.vector.reciprocal(rc[:], sm[:])
                nc.vector.tensor_scalar_mul(probs_pad[0:B, :], probs_pad[0:B, :],
                                            rc[:])

                ps_pm = rpsum.tile([1, E], F32)
                nc.tensor.matmul(ps_pm[:], quarter[:], probs_pad[:],
                                 start=True, stop=True)
                prow_pad = rmisc.tile([P, E], F32)
                nc.vector.memset(prow_pad[:], 0.0)
                nc.vector.tensor_copy(prow_pad[0:1, :], ps_pm[:])
                ps_pb = rpsum.tile([P, E], F32)
                nc.tensor.matmul(ps_pb[:], e0_ones[:], prow_pad[:],
                                 start=True, stop=True)
                nc.vector.tensor_copy(probs_bcast[:], ps_pb[:])

                # scaled identities for the merge matmuls
                for e in range(E):
                    nc.vector.tensor_scalar_mul(id_scaled[:, e, :], id_bf[:],
                                                probs_bcast[:, e:e + 1])
                # probs as a column vector (rows 0..7), for bias merges
                nc.vector.memset(probs_col[:], 0.0)
                tmp8 = rmisc.tile([E, E], F32)
                nc.vector.tensor_tensor(tmp8[:], probs_bcast[0:E, :],
                                        id_f32[0:E, 0:E], mybir.AluOpType.mult)
                nc.vector.reduce_sum(probs_col[0:E, :], tmp8[:],
                                     axis=mybir.AxisListType.X)

            # ================= weight merge (PE) + chunked AllGather ========
            with (
                tc.tile_pool(name="merge", bufs=12) as merge,
                tc.tile_pool(name="mev", bufs=4) as mev,
                tc.tile_pool(name="mpsum", bufs=2, space="PSUM") as mpsum,
            ):
                for j in range(DFF // 512):
                    win = merge.tile([P, E, 512], BF16)
                    nc.sync.dma_start(win[:], upw[j])
                    ps = mpsum.tile([P, 512], F32)
                    for e in range(E):
                        nc.tensor.matmul(ps[:], id_scaled[:, e, :], win[:, e, :],
                                         start=(e == 0), stop=(e == E - 1))
                    ev = mev.tile([P, 512], BF16)
                    nc.vector.tensor_copy(ev[:], ps[:])
                    nc.sync.dma_start(agu_in[j // 2][:, ds((j % 2) * 512, 512)],
                                      ev[:])
                    if j % 2 == 1:
                        nc.gpsimd.collective_compute(
                            "AllGather", mybir.AluOpType.bypass,
                            replica_groups=rg,
                            ins=[agu_in[j // 2][:]], outs=[agu_out[j // 2][:]],
                        )

                for fq in range(4):
                    for h in range(2):
                        win = merge.tile([P, E, 512], BF16)
                        nc.sync.dma_start(win[:], dnw[fq, h])
                        ps = mpsum.tile([P, 512], F32)
                        for e in range(E):
                            nc.tensor.matmul(ps[:], id_scaled[:, e, :],
                                             win[:, e, :],
                                             start=(e == 0), stop=(e == E - 1))
                        ev = mev.tile([P, 512], BF16)
                        nc.vector.tensor_copy(ev[:], ps[:])
                        nc.sync.dma_start(
                            agd_in[ds(fq * P, P), ds(h * 512, 512)], ev[:])

                nc.gpsimd.collective_compute(
                    "AllGather", mybir.AluOpType.bypass, replica_groups=rg,
                    ins=[agd_in[:]], outs=[agd_out[:]],
                )

            # ================= bias merges =================
            with (
                tc.tile_pool(name="bias", bufs=2) as bias,
                tc.tile_pool(name="bpsum", bufs=2, space="PSUM") as bpsum,
            ):
                for j in range(DFF // 512):
                    ub = bias.tile([P, 512], F32)
                    nc.vector.memset(ub[:], 0.0)
                    nc.sync.dma_start(ub[0:E, :], upb[:, ds(j * 512, 512)])
                    pb = bpsum.tile([1, 512], F32)
                    nc.tensor.matmul(pb[:], probs_col[:], ub[:],
                                     start=True, stop=True)
                    bev = bias.tile([1, 512], F32)
                    nc.vector.tensor_copy(bev[:], pb[:])
                    nc.sync.dma_start(bup_dram[None, ds(j * 512, 512)], bev[:])
                for j in range(D // 512):
                    db = bias.tile([P, 512], F32)
                    nc.vector.memset(db[:], 0.0)
                    nc.sync.dma_start(db[0:E, :], dnb[:, ds(j * 512, 512)])
                    pb = bpsum.tile([1, 512], F32)
                    nc.tensor.matmul(pb[:], probs_col[:], db[:],
                                     start=True, stop=True)
                    bev = bias.tile([1, 512], F32)
                    nc.vector.tensor_copy(bev[:], pb[:])
                    nc.sync.dma_start(bdn_dram[None, ds(j * 512, 512)], bev[:])

                nc.sync.dma_start(bup_sb[:],
                                  bup_dram.rearrange("(ot p) -> p ot", p=P))
                nc.sync.dma_start(bdn_sb[:],
                                  bdn_dram.rearrange("(dt p) -> p dt", p=P))

            # ================= GEMMs =================
            # o' layout (host-permuted): o'-tile 2t = linear f-tile t,
            # o'-tile 2t+1 = gated f-tile t.
            with tc.tile_pool(name="stpool", bufs=1) as stpool:
                st_all = stpool.tile([P, FKS, TPC], BF16)

                # ---- up GEMM + GLU -> st_all ----
                with (
                    tc.tile_pool(name="wstrip", bufs=3) as wstrip,
                    tc.tile_pool(name="glu", bufs=3) as glu,
                    tc.tile_pool(name="gpsum", bufs=2, space="PSUM") as gpsum,
                ):
                    for jc in range(NAG):
                        agv = agu_out[jc].rearrange("(kt p) o -> p kt o", p=P)
                        for h in range(2):
                            oc = 2 * jc + h
                            wsu = wstrip.tile([P, KS, 512], BF16)
                            nc.sync.dma_start(wsu[:], agv[:, :, ds(h * 512, 512)])
                            for j2 in range(2):
                                ft = 2 * oc + j2
                                lhs_l = wsu[:, :, ds(j2 * 256, P)]
                                lhs_g = wsu[:, :, ds(j2 * 256 + P, P)]
                                ps_l0 = gpsum.tile([P, 512], F32, tag="ps_l0")
                                ps_l1 = gpsum.tile([P, 512], F32, tag="ps_l1")
                                ps_g0 = gpsum.tile([P, 512], F32, tag="ps_g0")
                                ps_g1 = gpsum.tile([P, 512], F32, tag="ps_g1")
                                for kt in range(KS):
                                    nc.tensor.matmul(ps_l0[:], lhs_l[:, kt, :],
                                                     xt_bf[:, kt, 0:512],
                                                     start=(kt == 0),
                                                     stop=(kt == KS - 1))
                                    nc.tensor.matmul(ps_l1[:], lhs_l[:, kt, :],
                                                     xt_bf[:, kt, 512:1024],
                                                     start=(kt == 0),
                                                     stop=(kt == KS - 1))
                                for kt in range(KS):
                                    nc.tensor.matmul(ps_g0[:], lhs_g[:, kt, :],
                                                     xt_bf[:, kt, 0:512],
                                                     start=(kt == 0),
                                                     stop=(kt == KS - 1))
                                    nc.tensor.matmul(ps_g1[:], lhs_g[:, kt, :],
                                                     xt_bf[:, kt, 512:1024],
                                                     start=(kt == 0),
                                                     stop=(kt == KS - 1))
                                for th, ps_lx, ps_gx in ((0, ps_l0, ps_g0),
                                                         (1, ps_l1, ps_g1)):
                                    lin = glu.tile([P, 512], F32, tag="lin")
                                    nc.vector.tensor_scalar_add(
                                        lin[:], ps_lx[:],
                                        bup_sb[:, 2 * ft:2 * ft + 1])
                                    sil = glu.tile([P, 512], F32, tag="sil")
                                    nc.scalar.activation(
                                        sil[:], ps_gx[:],
                                        mybir.ActivationFunctionType.Silu,
                                        bias=bup_sb[:, 2 * ft + 1:2 * ft + 2],
                                        scale=1.0)
                                    nc.vector.tensor_tensor(
                                        st_all[:, ft, ds(th * 512, 512)],
                                        lin[:], sil[:], mybir.AluOpType.mult)

                # ---- down GEMM ----
                agd_view = agd_out.rearrange("(kt p) n -> p kt n", p=P)
                with (
                    tc.tile_pool(name="dwstrip", bufs=2) as dwstrip,
                    tc.tile_pool(name="opool", bufs=3) as opool,
                    tc.tile_pool(name="dpsum", bufs=2, space="PSUM") as dpsum,
                ):
                    for dp in range(4):
                        wd = dwstrip.tile([P, FKS, 256], BF16)
                        nc.sync.dma_start(wd[:], agd_view[:, :, ds(dp * 256, 256)])
                        for dtl in range(2):
                            dt = 2 * dp + dtl
                            ps_o0 = dpsum.tile([P, 512], F32, tag="ps_o0")
                            ps_o1 = dpsum.tile([P, 512], F32, tag="ps_o1")
                            for kt in range(FKS):
                                nc.tensor.matmul(ps_o0[:],
                                                 wd[:, kt, ds(dtl * P, P)],
                                                 st_all[:, kt, 0:512],
                                                 start=(kt == 0),
                                                 stop=(kt == FKS - 1))
                                nc.tensor.matmul(ps_o1[:],
                                                 wd[:, kt, ds(dtl * P, P)],
                                                 st_all[:, kt, 512:1024],
                                                 start=(kt == 0),
                                                 stop=(kt == FKS - 1))
                            for th, ps_ox in ((0, ps_o0), (1, ps_o1)):
                                ot = opool.tile([P, 512], F32, tag="ot")
                                nc.vector.tensor_scalar_add(
                                    ot[:], ps_ox[:], bdn_sb[:, dt:dt + 1])
                                nc.sync.dma_start(
                                    out[ds(dt * P, P), ds(th * 512, 512)], ot[:])

    nc.compile()
    return nc


def _get_nc():
    if "nc" not in _CACHED:
        _CACHED["nc"] = _build()
    return _CACHED["nc"]


def _prep(x, router_w, router_b, up_w, up_b, down_w, down_b):
    x = np.asarray(x, dtype=np.float32)
    router_w = np.asarray(router_w, dtype=np.float32)
    router_b = np.asarray(router_b, dtype=np.float32)
    up_w = np.asarray(up_w, dtype=np.float32)
    up_b = np.asarray(up_b, dtype=np.float32)
    down_w = np.asarray(down_w, dtype=np.float32)
    down_b = np.asarray(down_b, dtype=np.float32)

    bf = ml_dtypes.bfloat16

    # o-permutation: [L0, G0, L1, G1, ...] blocks of 128 rows
    perm = np.empty(DFF, dtype=np.int64)
    for t in range(F // P):
        perm[2 * t * P:(2 * t + 1) * P] = np.arange(t * P, (t + 1) * P)
        perm[(2 * t + 1) * P:(2 * t + 2) * P] = np.arange(F + t * P, F + (t + 1) * P)

    xf = x.reshape(T, D)
    up_bf = up_w.astype(bf)[:, perm, :]          # [E, DFF(perm), D]
    dn_bf = down_w.astype(bf)                    # [E, D, F]

    rwb = np.zeros((RWB_ROWS, E), dtype=np.float32)
    rwb[:D] = router_w.T
    rwb[D] = router_b
    upb_p = np.ascontiguousarray(up_b[:, perm])
    dnb_c = np.ascontiguousarray(down_b)

    in_maps = []
    for c in range(NCORES):
        xt_c = np.ascontiguousarray(xf[c * TPC:(c + 1) * TPC].T)
        # upw chunk-contiguous: [oc, p(d), e, o]
        upw_c = np.ascontiguousarray(
            up_bf[:, :, c * P:(c + 1) * P]
            .reshape(E, DFF // 512, 512, P).transpose(1, 3, 0, 2))
        # dnw chunk-contiguous: [fq, h, p(f), e, o(d')]
        dnw_c = np.ascontiguousarray(
            dn_bf[:, :, c * (F // NCORES):(c + 1) * (F // NCORES)]
            .transpose(0, 2, 1)                      # [E, 512(f), D]
            .reshape(E, 4, P, 2, 512).transpose(1, 3, 2, 0, 4))
        bsel_c = np.zeros((P, B), dtype=np.float32)
        bsel_c[:, c // 2] = 1.0
        in_maps.append({
            "xt": xt_c, "rwb": rwb, "upw": upw_c, "dnw": dnw_c,
            "upb": upb_p, "dnb": dnb_c, "bsel": bsel_c,
        })
    return in_maps


def kernel(x, router_w, router_b, up_w, up_b, down_w, down_b):
    in_maps = _prep(x, router_w, router_b, up_w, up_b, down_w, down_b)
    nc = _get_nc()
    res = run_bass_kernel_spmd(nc, in_maps, core_ids=list(range(NCORES)))

    outp = np.empty((T, D), dtype=np.float32)
    for c in range(NCORES):
        outp[c * TPC:(c + 1) * TPC] = res.results[c]["out"].T
    return outp.reshape(B, S, D)
